# revision 20
# baseline (speedup 1.0000x reference)
"""Trainium2 Bass kernel for nn_AudioNetwork (4-block STFT resonator chain).

Algorithm notes
---------------
Per block: frame x (win 2048, hop 1024), rfft, per-bin linear recurrence over
frames out_i = (spec_i + out_{i-1}) * tc, irfft, hann-windowed overlap-add,
tanh(gain * s).  Since every recurrence step multiplies by tc, bins with
tc == 0 never contribute: the (i)DFT only needs the nonzero bins of tc
(~10 of 1025 for the reference init).  Both transforms become tiny matmuls.

Device layout (per core, 4 batch elements):
  x is pre-transposed ON THE HOST into 8 DRAM tiles of
  (128 samples-in-chunk, 1024 cols) fp16, where col = batch*256 + chunk.
  In this layout both the forward DFT (contract over the 1024 samples of a
  hop-chunk) and the inverse DFT (produce samples) are natural PE matmuls
  with no device-side transposes; block outputs y_k are shipped back in the
  same layout and the mixer-weighted sum + un-transpose run on the host.

  Forward: spec_i needs frame i = [chunk_i, chunk_{i+1}] but
  cos/sin(2*pi*k*(s+1024)/2048) = (-1)^k * cos/sin(2*pi*k*s/2048), so the
  full recurrence input in1_i = U^T x_i + sign (.) U^T x_{i+1} is produced
  entirely on the PE: a second stationary (U * sign) runs over the
  column-shifted moving operand and accumulates into the same PSUM region.
  The recurrence itself is a DVE tensor_tensor_scan per batch (fp32 state,
  fp16 output, reading in1 straight from PSUM).  Overlap-add is folded into
  the inverse matmul by stacking [outs; outs_shifted_one_frame] as the
  moving operand (the shift is a small SBUF-to-SBUF DMA; shifted rows live
  at partition 64).  All DFT matmuls run in fp16 (1 col/cycle on PE); tanh
  runs on ACT straight out of PSUM with the gain folded into the activation
  scale, writing fp16.
"""

import numpy as np
from contextlib import ExitStack

import concourse.bass as bass
import concourse.tile as tile
from concourse import bacc, mybir
from concourse import bass_utils

F32 = mybir.dt.float32
F16 = mybir.dt.float16
F16_NP = np.float16
WS = 2048
STEP = 1024
NCOEF = WS // 2 + 1
NBLK = 4
B = 32
T = 262144
NCORES = 8
BL = B // NCORES          # batch per core
NF = T // STEP            # 256 frames/chunks
KT = STEP // 128          # 8 K-tiles of the forward contraction
COLS = BL * NF            # 1024 free columns (batch-major)
MAX_BINS_PER_CHUNK = 32

_CACHE = {}


def _plan_chunks(tc_vec):
    nz = np.nonzero(tc_vec)[0]
    if len(nz) == 0:
        nz = np.array([1], dtype=np.int64)  # dummy bin with tc=0: contributes 0
    chunks = [nz[i:i + MAX_BINS_PER_CHUNK] for i in range(0, len(nz), MAX_BINS_PER_CHUNK)]
    return chunks


def _host_matrices(tc_vec, chunks):
    """Build per-chunk constant arrays (float64 math, fp16/f32 storage)."""
    hann = 0.5 - 0.5 * np.cos(2.0 * np.pi * np.arange(WS) / WS)
    out = []
    for bins in chunks:
        nb = len(bins)
        k = bins.astype(np.float64)
        tcv = tc_vec[bins].astype(np.float64)
        s = np.arange(STEP, dtype=np.float64)
        ang = 2.0 * np.pi * np.outer(s, k) / WS                      # (1024, nb)
        # forward matrix padded to 64 rows: the matmul then writes exact
        # zeros into rows 2nb:64, so no memset is needed downstream
        bf = np.zeros((STEP, 64))
        bf[:, 0:nb] = np.cos(ang) * tcv
        bf[:, nb:2 * nb] = -np.sin(ang) * tcv
        sign = np.zeros(64); sign[0:nb] = (-1.0) ** k; sign[nb:2 * nb] = (-1.0) ** k
        bfs = bf * sign                                              # second window half
        bf_t = bf.reshape(KT, 128, 64).transpose(1, 0, 2)            # (128, 8, 64)
        bfs_t = bfs.reshape(KT, 128, 64).transpose(1, 0, 2)
        tcrep = np.zeros((64, NF)); tcrep[0:nb] = tcv[:, None]; tcrep[nb:2 * nb] = tcv[:, None]
        w = np.where((bins == 0) | (bins == WS // 2), 1.0, 2.0)
        s2 = np.arange(WS, dtype=np.float64)
        ang2 = 2.0 * np.pi * np.outer(k, s2) / WS                    # (nb, 2048)
        are = (w[:, None] / WS) * np.cos(ang2) * hann
        aim = -(w[:, None] / WS) * np.sin(ang2) * hann
        w1 = np.concatenate([are[:, :STEP], aim[:, :STEP]], axis=0)  # (2nb, 1024) cur frame
        w2 = np.concatenate([are[:, STEP:], aim[:, STEP:]], axis=0)  # (2nb, 1024) prev frame
        # rows 2nb:64 (and 64+2nb:128) are zero: scb carries 64-row
        # current-frame and 64-row shifted operands
        pad = np.zeros((64 - 2 * nb, WS // 2))
        winv = np.concatenate([w1, pad, w2, pad], axis=0).reshape(128, KT, 128)
        out.append(dict(
            nb=nb,
            bf=np.ascontiguousarray(bf_t.astype(F16_NP)),
            bfs=np.ascontiguousarray(bfs_t.astype(F16_NP)),
            winv=np.ascontiguousarray(winv.astype(F16_NP)),
            tcrep=np.ascontiguousarray(tcrep, dtype=np.float32),
        ))
    return out


def _build(chunk_sizes, gains):
    """Trace+compile the Bass program. chunk_sizes: tuple of tuples of nb per block."""
    nc = bacc.Bacc("TRN2", target_bir_lowering=False, debug=False)
    xt_d = nc.dram_tensor("xt", (KT, 128, BL, NF), F16, kind="ExternalInput").ap()
    ys_d = nc.dram_tensor("ys", (NBLK, 128, KT, BL, NF), F16, kind="ExternalOutput").ap()
    cons = {}
    for kb in range(NBLK):
        for c, nb in enumerate(chunk_sizes[kb]):
            cons[f"wc_{kb}_{c}"] = nc.dram_tensor(f"wc_{kb}_{c}", (128, KT, 256), F16, kind="ExternalInput").ap()
            cons[f"tr_{kb}_{c}"] = nc.dram_tensor(f"tr_{kb}_{c}", (64, NF), F32, kind="ExternalInput").ap()

    mult = mybir.AluOpType.mult
    add = mybir.AluOpType.add

    with tile.TileContext(nc) as tc, ExitStack() as ctx:
        cpool = ctx.enter_context(tc.tile_pool(name="const", bufs=1))
        big = ctx.enter_context(tc.tile_pool(name="big", bufs=1))
        work = ctx.enter_context(tc.tile_pool(name="work", bufs=2))
        # PSUM budget (8 banks): uv 2x2 + ips 2x2 = 8
        upool = ctx.enter_context(tc.tile_pool(name="upool", bufs=2, space="PSUM"))
        ppool = ctx.enter_context(tc.tile_pool(name="ppool", bufs=2, space="PSUM"))

        # 5 resident signal tiles: x, y1..y4 (fp16, 16KB/partition each)
        sig = [big.tile([128, KT, BL, NF], F16, tag=f"sig{i}", name=f"sig{i}")
               for i in range(NBLK + 1)]

        # ---- load x first (already transposed+fp16 on host) ----
        for a in range(KT):
            eng = nc.sync if a % 2 == 0 else nc.gpsimd
            eng.dma_start(sig[0][:, a], xt_d[a])

        # constants (merged per chunk: one fp16 DMA bf|bs|wi, one fp32 tr)
        bf_t, bs_t, wi_t, tr_t = {}, {}, {}, {}
        for kb in range(NBLK):
            for c, nb in enumerate(chunk_sizes[kb]):
                wc = cpool.tile([128, KT, 256], F16, tag=f"wc{kb}_{c}", name=f"wc{kb}_{c}")
                eng = nc.sync if kb % 2 == 0 else nc.gpsimd
                eng.dma_start(wc[:], cons[f"wc_{kb}_{c}"][:])
                bf_t[(kb, c)] = wc[:, :, 0:64]
                bs_t[(kb, c)] = wc[:, :, 64:128]
                wi_t[(kb, c)] = wc[:, :, 128:256]
                tr_t[(kb, c)] = cpool.tile([64, NF], F32, tag=f"tr{kb}_{c}", name=f"tr{kb}_{c}")
                eng2 = nc.gpsimd if kb % 2 == 0 else nc.sync
                eng2.dma_start(tr_t[(kb, c)], cons[f"tr_{kb}_{c}"][:])

        # ---- block chain ----
        for kb in range(NBLK):
            src = sig[kb]
            dst = sig[kb + 1]
            sizes = chunk_sizes[kb]
            scb_list = []
            for c, nb in enumerate(sizes):
                # forward DFT + second-window-half combine, all in PSUM:
                # in1[:, b, i] = bf^T x[b, i] + (bf*sign)^T x[b, i+1]
                uvg = [upool.tile([64, 2, NF], F32, tag=f"uv{g}", name=f"uv{g}")
                       for g in range(2)]
                for g in (1, 0):
                    uv = uvg[g]
                    for a in range(KT):
                        nc.tensor.matmul(uv[:], bf_t[(kb, c)][:, a],
                                         src[:, a, 2 * g:2 * g + 2, :],
                                         start=(a == 0), stop=False)
                    for a in range(KT):
                        nc.tensor.matmul(uv[:, :, 0:NF - 1],
                                         bs_t[(kb, c)][:, a],
                                         src[:, a, 2 * g:2 * g + 2, 1:NF],
                                         start=False, stop=(a == KT - 1))
                if c == 0 and kb > 0:
                    # ship y_{kb-1} now on the scalar queue: the waits are
                    # already satisfied there (tanh ran on ACT), descriptors
                    # drain during fwd, and sync/gpsimd stay clear for the
                    # critical shift copies
                    for m in range(KT):
                        nc.scalar.dma_start(ys_d[kb - 1][:, m], src[:, m])
                scb = [work.tile([128, 2, NF], F16, tag=f"scb{g}", name=f"scb{g}",
                                 bufs=2 * len(sizes)) for g in range(2)]
                for g in (1, 0):
                    nc.gpsimd.memset(scb[g][64:128, :, 0:1], 0.0)
                for b in (2, 3, 0, 1):
                    sc = scb[b // 2]
                    # out_i = (out_{i-1} + in1_i) * tc, fp32 state, fp16 out
                    nc.vector.tensor_tensor_scan(
                        sc[0:64, b % 2, :], tr_t[(kb, c)], uvg[b // 2][:, b % 2, :],
                        initial=0.0, op0=mult, op1=add)
                    # prev-frame rows at partition 64 (cross-partition: DMA)
                    eng = nc.sync if b % 2 == 0 else nc.gpsimd
                    eng.dma_start(sc[64:128, b % 2, 1:NF], sc[0:64, b % 2, 0:NF - 1])
                scb_list.append(scb)
            # inverse DFT + hann + OLA (+ chunk accumulation in PSUM)
            # g-outer with per-(m,g) PSUM tiles: the g1 lane's tanh halves
            # unblock the next block's g1 forward while g0 still computes
            for g in (1, 0):
                for m in range(KT):
                    ps = ppool.tile([128, 2, NF], F32, tag=f"ips{g}", name=f"ips{g}")
                    for c in range(len(sizes)):
                        nc.tensor.matmul(ps[:],
                                         wi_t[(kb, c)][:, m],
                                         scb_list[c][g][:, :, :],
                                         start=(c == 0), stop=(c == len(sizes) - 1))
                    nc.scalar.activation(dst[:, m, 2 * g:2 * g + 2, :], ps[:],
                                         mybir.ActivationFunctionType.Tanh,
                                         scale=float(gains[kb]))
                    if kb == NBLK - 1 and g == 0:
                        # last block: ship per m-tile to hide the output tail
                        eng = nc.sync if m % 2 == 0 else nc.gpsimd
                        eng.dma_start(ys_d[kb][:, m], dst[:, m])

    nc.compile()
    return nc


def prepare(x, transfers, gains, mixer):
    """Compile (cached) and build per-core input maps."""
    x = np.asarray(x, dtype=np.float32)
    transfers = np.asarray(transfers, dtype=np.float32)
    gains = np.asarray(gains, dtype=np.float64)

    plans = [_plan_chunks(transfers[kb]) for kb in range(NBLK)]
    chunk_sizes = tuple(tuple(len(ch) for ch in pl) for pl in plans)
    key = (chunk_sizes, tuple(np.round(gains, 9)))
    if key not in _CACHE:
        _CACHE[key] = _build(chunk_sizes, gains)
    nc = _CACHE[key]

    const_map = {}
    for kb in range(NBLK):
        mats = _host_matrices(transfers[kb].astype(np.float64), plans[kb])
        for c, md in enumerate(mats):
            const_map[f"wc_{kb}_{c}"] = np.ascontiguousarray(
                np.concatenate([md["bf"], md["bfs"], md["winv"]], axis=2))
            const_map[f"tr_{kb}_{c}"] = md["tcrep"]

    # host-side transpose: (BL, NF, KT, 128) -> (KT, 128, BL, NF), fp16
    xr = x.reshape(B, T).astype(F16_NP)
    in_maps = []
    for core in range(NCORES):
        m = dict(const_map)
        xc = xr[core * BL:(core + 1) * BL].reshape(BL, NF, KT, 128)
        m["xt"] = np.ascontiguousarray(xc.transpose(2, 3, 0, 1))
        in_maps.append(m)
    return nc, in_maps


def postprocess(res, x, mixer):
    mixer = np.asarray(mixer, dtype=np.float64)
    wm = np.exp(mixer - mixer.max())
    wm = wm / wm.sum()
    outs = []
    for i in range(NCORES):
        ys = np.asarray(res.results[i]["ys"]).astype(np.float32)   # (NBLK, 128, KT, BL, NF)
        ymix = np.tensordot(wm[1:].astype(np.float32), ys, axes=(0, 0))  # (128, KT, BL, NF)
        yt = ymix.transpose(2, 3, 1, 0).reshape(BL, 1, T)
        outs.append(yt)
    out = np.concatenate(outs, axis=0)
    out += np.float32(wm[0]) * np.asarray(x, dtype=np.float32).reshape(B, 1, T)
    return out.astype(np.float32)


def kernel(x, transfers, gains, mixer):
    nc, in_maps = prepare(x, transfers, gains, mixer)
    res = bass_utils.run_bass_kernel_spmd(nc, in_maps, core_ids=list(range(NCORES)))
    return postprocess(res, x, mixer)
